# revision 57
# baseline (speedup 1.0000x reference)
"""Causal dot-product attention (B=2, H=16, S=2048, D=64, fp32) on 8 NeuronCores.

Sharding: the 32 (batch, head) slices are split 4-per-core. Each head is
computed flash-attention style but transposed: scores are built as
S^T[k, q] = K_tile @ Q^T so that exp(S^T) lands in SBUF already in the
[k-partition, q-free] layout the PV matmul needs as its moving operand —
no on-chip transposes anywhere. The softmax denominator rides along as a
ones-column appended to V (output row 64), and the final divide + layout
transpose happen on the host.
"""

import numpy as np

B, H, S, D = 2, 16, 2048, 64
N_CORES = 8
HPC = (B * H) // N_CORES  # heads per core = 4
PAIRS = HPC // 2          # head pairs per core = 2
QB = 512                  # query block (free dim of the S^T matmul)
KT = 128                  # key tile (partition dim of S^T)
NQB = S // QB             # 4
NKT = S // KT             # 16
VC = D + 1                # V columns + ones column = 65
STRIP = 1536              # PSUM strip width (3 banks): 3 full or 4 diagonal tiles
SCALE = 1.0 / 8.0         # 1/sqrt(D)

_CACHE = {}


def _build():
    import concourse.mybir as mybir
    import concourse.tile as tile
    from concourse import bacc

    f32 = mybir.dt.float32
    f32r = mybir.dt.float32r
    nc = bacc.Bacc("TRN2")

    qt_d = nc.dram_tensor("qt", [PAIRS, 128, S], f32r, kind="ExternalInput")
    kt_d = nc.dram_tensor("kt", [PAIRS, 128, S], f32r, kind="ExternalInput")
    v_d = nc.dram_tensor("v", [PAIRS, 128, 2 * NKT * VC], f32r, kind="ExternalInput")
    out_d = nc.dram_tensor("out", [HPC, NQB, VC, QB], f32, kind="ExternalOutput")

    qt_ap = qt_d.ap()
    kt_ap = kt_d.ap()
    v_ap = v_d.ap()
    out_ap = out_d.ap()

    with tile.TileContext(nc) as tc:
        with (
            tc.tile_pool(name="const", bufs=1) as constp,
            tc.tile_pool(name="inp", bufs=1) as inp,
            tc.tile_pool(name="pt", bufs=8) as ptp,
            tc.tile_pool(name="ob", bufs=2) as obp,
            tc.tile_pool(name="st", bufs=2, space="PSUM") as stp,
            tc.tile_pool(name="ops", bufs=2, space="PSUM") as opsp,
        ):
            # Causal masking runs on the PE as a -1e9 accumulate-matmul over
            # the first 128 columns of each diagonal slot: (L.T @ R)[p, j]
            # = -1e9 iff j < p, with L[c, p] = 1 iff p >= c and
            # R[c, j] = -1e9 iff j == c - 1. bf16 keeps it at 1 cyc/row.
            bf16 = mybir.dt.bfloat16
            warm_t = constp.tile([64, KT], bf16)
            nc.gpsimd.memset(warm_t[:], 0.5)
            lmask = constp.tile([128, KT], bf16)
            nc.gpsimd.memset(lmask[:], 1.0)
            nc.gpsimd.affine_select(
                out=lmask[:],
                in_=lmask[:],
                compare_op=mybir.AluOpType.is_ge,
                fill=0.0,
                base=0,
                pattern=[[1, KT]],
                channel_multiplier=-1,
            )
            rmask = constp.tile([128, KT], bf16)
            nc.gpsimd.memset(rmask[:], -1e9)
            nc.gpsimd.affine_select(
                out=rmask[:],
                in_=rmask[:],
                compare_op=mybir.AluOpType.is_ge,
                fill=0.0,
                base=1,
                pattern=[[1, KT]],
                channel_multiplier=-1,
            )
            nc.gpsimd.affine_select(
                out=rmask[:],
                in_=rmask[:],
                compare_op=mybir.AluOpType.is_ge,
                fill=0.0,
                base=-1,
                pattern=[[-1, KT]],
                channel_multiplier=1,
            )

            qt_sbs, kt_sbs, v_sbs = [], [], []
            for pair in range(PAIRS):
                qt_sb = inp.tile([128, S], f32r, tag=f"qt{pair}")
                kt_sb = inp.tile([128, S], f32r, tag=f"kt{pair}")
                v_sb = inp.tile([128, 2 * NKT * VC], f32r, tag=f"v{pair}")
                qt_sbs.append(qt_sb)
                kt_sbs.append(kt_sb)
                v_sbs.append(v_sb)
                # chunked loads so the first compute block starts early;
                # qb loop runs descending so Q chunks load high-to-low
                for sl, qsl in [
                    (slice(0, 512), slice(0, 512)),
                    (slice(512, 1024), slice(1536, 2048)),
                    (slice(1024, 1536), slice(1024, 1536)),
                    (slice(1536, 2048), slice(512, 1024)),
                ]:
                    nc.sync.dma_start(kt_sb[:, sl], kt_ap[pair, :, sl])
                    nc.sync.dma_start(qt_sb[:, qsl], qt_ap[pair, :, qsl])
                for h2 in range(2):
                    for i in range(4):
                        vsl = slice(
                            (h2 * NKT + i * 4) * VC, (h2 * NKT + (i + 1) * 4) * VC
                        )
                        nc.sync.dma_start(v_sb[:, vsl], v_ap[pair, :, vsl])

            # Flat strip stream across all (head, q-block) pairs, emitted
            # with one-strip lookahead: strip g+1's score matmuls precede
            # strip g's exp/PV in program order, so the PE never blocks the
            # next strip behind a PV that is waiting on the ScalarE.
            strip_list = []  # (h, qb, slots, spans, new_block, end_block)
            for h in range(HPC):
                for qb in [0, 3, 2, 1]:
                    d = 4 * qb
                    # the 4 diagonal tiles pack into one strip, ordered so no
                    # matmul output crosses a 512-col PSUM bank: widths
                    # 512/384/128/256 at offsets 0/512/896/1024 (contiguous)
                    slots = [
                        (d + 0, 0, QB),
                        (d + 1, QB, QB - KT),
                        (d + 3, 896, KT),
                        (d + 2, 1024, QB - 2 * KT),
                    ]
                    diag_group = (slots, [(0, 1280)])
                    groups = []
                    # full tiles in strips of up to 3, remainder first so
                    # short ACT ops land where the PE is building runway
                    sizes = {0: [], 1: [2, 2], 2: [2, 3, 3], 3: [3, 3, 3, 3]}[qb]
                    kt0 = 0
                    for n in sizes:
                        chunk = list(range(kt0, kt0 + n))
                        kt0 += n
                        groups.append(
                            (
                                [(kt, j * QB, QB) for j, kt in enumerate(chunk)],
                                [(0, n * QB)],
                            )
                        )
                    groups.append(diag_group)
                    for gi, (slots, spans) in enumerate(groups):
                        strip_list.append(
                            (h, qb, slots, spans, gi == 0, gi == len(groups) - 1)
                        )

            def emit_scores(s, warmup=False):
                h, qb, slots, spans, new_block, end_block = s
                pair, h2 = divmod(h, 2)
                qt_sb, kt_sb = qt_sbs[pair], kt_sbs[pair]
                p0 = 64 * h2
                qs = qb * QB
                o_ps = opsp.tile([VC, QB], f32, tag="o", name="o_ps") if new_block else None
                st = stp.tile([128, STRIP], f32, tag="st")
                pt = ptp.tile([128, STRIP], f32r, tag="pt")
                if warmup:
                    # spin the PE on const data while input DMAs land, so the
                    # HAM clock gate is already released (2.4 GHz) when the
                    # first real matmuls arrive; the first real slot's
                    # start=True clears this junk from PSUM
                    for _ in range(24):
                        nc.tensor.matmul(
                            st[:, :KT], warm_t[:], warm_t[:], start=True, stop=True
                        )
                for kt, off, w in slots:
                    diag = w < QB or kt == 4 * qb
                    nc.tensor.matmul(
                        st[:, off : off + w],
                        kt_sb[p0 : p0 + 64, kt * KT : kt * KT + KT],
                        qt_sb[p0 : p0 + 64, qs + QB - w : qs + QB],
                        start=True,
                        stop=not diag,
                    )
                    if diag:
                        # causal triangle only occupies the slot's first
                        # 128 columns (col >= 128 > any partition index)
                        nc.tensor.matmul(
                            st[:, off : off + KT],
                            lmask[:],
                            rmask[:],
                            start=False,
                            stop=True,
                            skip_group_check=True,
                        )
                return st, pt, o_ps

            o_cur = None
            def finish_strip(s, tiles):
                nonlocal o_cur
                h, qb, slots, spans, new_block, end_block = s
                pair, h2 = divmod(h, 2)
                v_sb = v_sbs[pair]
                st, pt, o_ps = tiles
                if new_block:
                    o_cur = o_ps
                first_kt = slots[0][0] if new_block else None
                for s0, s1 in spans:
                    nc.scalar.activation(
                        pt[:, s0:s1],
                        st[:, s0:s1],
                        mybir.ActivationFunctionType.Exp,
                        scale=SCALE,
                    )
                for i, (kt, off, w) in enumerate(slots):
                    vs = (h2 * NKT + kt) * VC
                    nc.tensor.matmul(
                        o_cur[:, QB - w :],
                        v_sb[:, vs : vs + VC],
                        pt[:, off : off + w],
                        start=(new_block and i == 0),
                        stop=(end_block and i == len(slots) - 1),
                    )
                if end_block:
                    o_sb = obp.tile([VC, QB], f32, tag="o_sb")
                    nc.vector.tensor_copy(o_sb[:], o_cur[:])
                    nc.sync.dma_start(out_ap[h, qb], o_sb[:])

            pending = []
            for si, s in enumerate(strip_list):
                tiles = emit_scores(s, warmup=(si == 0))
                pending.append((s, tiles))
                if len(pending) > 3:
                    finish_strip(*pending.pop(0))
            for p in pending:
                finish_strip(*p)
    nc.compile()
    return nc


def kernel(Q, K, V, padding_mask, attention_mask):
    """Full-input entry point: shards heads across 8 cores internally.

    padding_mask is all-True and attention_mask is the causal tril for this
    module config; causality is implemented directly in the device kernel.
    """
    try:  # absent in slim containers; run_bass_kernel_spmd imports it when
        import antenv.axon_hooks  # noqa: F401  # BASS_TRACE is set
    except ImportError:
        import sys as _sys
        import types as _types

        _m = _types.ModuleType("antenv.axon_hooks")
        _m.get_axon_ntff_profile_hook = lambda: None
        _sys.modules["antenv.axon_hooks"] = _m

    from concourse.bass_utils import run_bass_kernel_spmd

    if "nc" not in _CACHE:
        _CACHE["nc"] = _build()
    nc = _CACHE["nc"]

    Qh = np.asarray(Q, dtype=np.float32).reshape(B * H, S, D)
    Kh = np.asarray(K, dtype=np.float32).reshape(B * H, S, D)
    Vh = np.asarray(V, dtype=np.float32).reshape(B * H, S, D)

    in_maps = []
    for c in range(N_CORES):
        sl = slice(c * HPC, (c + 1) * HPC)
        # [HPC, S, D] -> [HPC, D, S] -> [PAIRS, 128, S]
        qt = np.ascontiguousarray(Qh[sl].transpose(0, 2, 1)).reshape(PAIRS, 128, S)
        kt = np.ascontiguousarray(Kh[sl].transpose(0, 2, 1)).reshape(PAIRS, 128, S)
        # V + ones column: [HPC, S, VC] -> [PAIRS, 2, NKT, 128, VC]
        vv = np.concatenate(
            [Vh[sl], np.ones((HPC, S, 1), dtype=np.float32)], axis=-1
        ).reshape(PAIRS, 2, NKT, 128, VC)
        # -> [PAIRS, 128(p), 2(h2), NKT, VC]
        vv = np.ascontiguousarray(vv.transpose(0, 3, 1, 2, 4)).reshape(
            PAIRS, 128, 2 * NKT * VC
        )
        in_maps.append({"qt": qt, "kt": kt, "v": vv})

    res = run_bass_kernel_spmd(nc, in_maps, core_ids=list(range(N_CORES)))
    kernel.last_results = res

    out = np.empty((B * H, S, D), dtype=np.float32)
    for c in range(N_CORES):
        o = res.results[c]["out"]  # [HPC, NQB, VC, QB]
        num = o[:, :, :D, :]      # [HPC, NQB, D, QB]
        den = o[:, :, D:, :]      # [HPC, NQB, 1, QB]
        oc = (num / den).transpose(0, 1, 3, 2).reshape(HPC, S, D)
        out[c * HPC : (c + 1) * HPC] = oc
    return out.reshape(B, H, S, D)
